# revision 3
# baseline (speedup 1.0000x reference)
"""CrossConv2d (concat -> 3x3 conv -> BN -> +skip -> ReLU) on 8 Trainium2 cores.

Data-parallel over (b*s)=32: 4 images per core, all sharing one u (same b).
v2 design vs the fp32r baseline:
  - bf16 everywhere (inputs, weights, outputs): FWL weight loads overlap with
    matmul streaming (fp32r must self-load serially, ~128cy/matmul), and HBM
    traffic halves. PSUM accumulation stays fp32.
  - u-sharing: the conv is linear in the concat input, so the 64 u-channel
    contribution y_u (incl. u's skip identity) is computed ONCE per core and
    added per-image via VectorE, cutting per-image contraction to 64 channels.
  - tap-pair packing: per-image 9 taps of K=64 are packed into 4 K=128
    matmuls + 1 K=64 matmul per 512-px chunk using two SBUF layouts per
    image: X1 = [v@0 ; v@+1col] and X2 = [v@0 ; v@+1row]; X2 is built from
    X1 by VectorE copies (in-partition shifts), not extra HBM reads.
  => tensor work/core: 5 image-passes x 5 matmuls/chunk ~ 416k cy ~ 173us
     vs baseline 9 matmuls/chunk x 4 images + weight-load stalls ~ 302us.
"""

import numpy as np
import ml_dtypes

import concourse.bacc as bacc
import concourse.mybir as mybir
from concourse import tile
from concourse.bass_utils import run_bass_kernel_spmd

EPS = 1e-5
BF16 = ml_dtypes.bfloat16

B, S, C1, C2, H, W = 4, 8, 64, 64, 128, 128
CC = C1 + C2
N_CORES = 8
IMG = (B * S) // N_CORES  # 4
WP, HP = W + 2, H + 2     # padded width/height
NPX = HP * WP             # padded image pixels (16900)
NQ = H * WP               # output columns incl. junk pad cols (16640)
XW = NPX + 8              # X tile width (max col read = 16901)
CHUNK = 512               # one PSUM bank
NBLK = 16                 # DMA blocks per image half

F32 = mybir.dt.float32
BF = mybir.dt.bfloat16

# lhsT slots: (tap_a, tap_b|None, use_X2, rhs col offset)
# tap (dy,dx) at out q reads input flat q + off - 1 in lower-half layout;
# X1 upper = lower shifted +1 col, X2 upper = lower shifted +1 row (WP cols).
SLOTS = [
    ((-1, -1), (-1, 0), False, 0),
    ((0, -1), (0, 0), False, WP),       # center tap in upper -> skip identity
    ((1, -1), (1, 0), False, 2 * WP),
    ((-1, 1), (0, 1), True, 2),
    ((1, 1), None, True, 2 * WP + 2),   # single, K=64
]

_CACHE = {}


def _build_program():
    nc = bacc.Bacc(
        "TRN2", target_bir_lowering=False, debug=False, num_devices=N_CORES
    )
    u_d = nc.dram_tensor("u", [C1, NPX], BF, kind="ExternalInput")
    v_d = nc.dram_tensor("v", [IMG, C2, NPX], BF, kind="ExternalInput")
    wu_d = nc.dram_tensor("wu", [CC, 5 * CC], BF, kind="ExternalInput")
    wv_d = nc.dram_tensor("wv", [CC, 5 * CC], BF, kind="ExternalInput")
    sh_d = nc.dram_tensor("shift", [CC, 1], F32, kind="ExternalInput")
    o_d = nc.dram_tensor("o", [IMG, CC, NQ], BF, kind="ExternalOutput")

    blk = [(NPX * k // NBLK, NPX * (k + 1) // NBLK) for k in range(NBLK)]
    starts = [CHUNK * k for k in range(32)] + [32 * CHUNK]
    chunks = [(st, min(st + CHUNK, NQ)) for st in starts]

    with tile.TileContext(nc) as tc:
        with (
            tc.tile_pool(name="consts", bufs=1) as cpool,
            tc.tile_pool(name="ostrip", bufs=6) as opool,
            tc.tile_pool(name="psum", bufs=8, space="PSUM") as ppool,
        ):
            xa1 = cpool.tile([CC, XW], BF)
            xa2 = cpool.tile([CC, XW], BF)
            xb1 = cpool.tile([CC, XW], BF)
            xb2 = cpool.tile([CC, XW], BF)
            yu_sb = cpool.tile([CC, NQ], BF)
            wu_sb = cpool.tile([CC, 5 * CC], BF)
            wv_sb = cpool.tile([CC, 5 * CC], BF)
            sh_sb = cpool.tile([CC, 1], F32)

            # consts first: first matmul group only needs wu slot 0 + xa1 head
            nc.scalar.dma_start(wu_sb[:], wu_d[:])
            nc.scalar.dma_start(wv_sb[:], wv_d[:])
            nc.scalar.dma_start(sh_sb[:], sh_d[:])

            def fill_pads(x1, src_d):
                # lower col 0 and tail junk cols (read only by single-E at
                # q=16639); src row 0 of the padded image is all zeros.
                nc.scalar.dma_start(x1[0:64, 0:1], src_d[:, 0:1])
                nc.scalar.dma_start(x1[64:CC, NPX : NPX + 1], src_d[:, 0:1])
                nc.scalar.dma_start(x1[0:64, 1 + NPX : XW], src_d[:, 0 : XW - NPX - 1])

            def load_half(x1, j0, j1, src_lo, src_hi):
                # X1 lower: image at col 1+p ; X1 upper: image at col p
                nc.sync.dma_start(x1[0:64, 1 + j0 : 1 + j1], src_lo[:, j0:j1])
                nc.sync.dma_start(x1[64:CC, j0:j1], src_hi[:, j0:j1])

            def make_x2_lo(x1, x2, j0, j1):
                # X2 lower = X1 lower (identity, incl. pad cols on block 0 /
                # tail) — pure in-block copy.
                lo0, lo1 = (0 if j0 == 0 else 1 + j0), (XW if j1 == NPX else 1 + j1)
                nc.vector.tensor_copy(x2[0:64, lo0:lo1], x1[0:64, lo0:lo1])

            def make_x2_hi(x1, x2, j0, j1):
                # X2 upper col j = image flat j-1+WP = X1 upper col j+WP-1;
                # reads spill into the NEXT block, so callers issue this one
                # block behind the DMA.
                hi1 = min(j1, NPX + 2 - WP)
                if j0 < hi1:
                    nc.vector.tensor_copy(
                        x2[64:CC, j0:hi1], x1[64:CC, j0 + WP - 1 : hi1 + WP - 1]
                    )

            def load_image(x1, x2, src_d):
                fill_pads(x1, src_d)
                for j, (j0, j1) in enumerate(blk):
                    load_half(x1, j0, j1, src_d, src_d)
                    make_x2_lo(x1, x2, j0, j1)
                    if j > 0:
                        make_x2_hi(x1, x2, *blk[j - 1])
                make_x2_hi(x1, x2, *blk[-1])

            def conv_pass(x1, x2, w_sb, emit):
                """5-matmul conv over all chunks; emit(ps, c0, c1, gi, g0, g1)
                drains one chunk; gi = index of chunk in its group of 3."""
                for g0 in range(0, len(chunks), 4):
                    grp = chunks[g0 : g0 + 4]
                    pss = {}
                    for c0, _ in grp:
                        ps_g = ppool.tile([CC, CHUNK], F32, tag="ps")
                        pss[c0] = ps_g
                    for k, (_, tb, use_x2, off) in enumerate(SLOTS[:4]):
                        x = x2 if use_x2 else x1
                        for c0, c1 in grp:
                            nc.tensor.matmul(
                                pss[c0][0:CC, 0 : c1 - c0],
                                w_sb[0:CC, k * CC : (k + 1) * CC],
                                x[0:CC, c0 + off : c1 + off],
                                start=(k == 0),
                                stop=False,
                            )
                    off_e = SLOTS[4][3]
                    for ei, (c0, c1) in enumerate(grp):
                        if ei % 2 == 0:
                            nc.tensor.matmul(
                                pss[c0][0:CC, 0 : c1 - c0],
                                w_sb[0:64, 4 * CC : 5 * CC],
                                x2[0:64, c0 + off_e : c1 + off_e],
                                start=False,
                                stop=True,
                            )
                        else:
                            nc.tensor.matmul(
                                pss[c0][0:CC, 0 : c1 - c0],
                                w_sb[64:CC, 4 * CC : 5 * CC],
                                x2[64:CC, c0 + off_e - WP : c1 + off_e - WP],
                                start=False,
                                stop=True,
                            )
                    for gi, (c0, c1) in enumerate(grp):
                        emit(pss[c0], c0, c1, gi, grp[0][0], grp[-1][1])

            # ---- phase 0: y_u from U tiles (xa slots) ----
            load_image(xa1, xa2, u_d)

            def emit_yu(ps, c0, c1, gi, g0, g1):
                nc.scalar.activation(
                    yu_sb[:, c0:c1],
                    ps[:, 0 : c1 - c0],
                    mybir.ActivationFunctionType.Copy,
                )

            conv_pass(xa1, xa2, wu_sb, emit_yu)

            # ---- per-image passes ----
            for i in range(IMG):
                xs1, xs2 = (xb1, xb2) if i % 2 == 0 else (xa1, xa2)
                load_image(xs1, xs2, v_d[i])

                last_img = i == IMG - 1
                oeng = nc.sync if last_img else nc.gpsimd
                ostate = {}

                def emit_img(ps, c0, c1, gi, g0, g1, i=i, last_img=last_img,
                             oeng=oeng, ostate=ostate):
                    n = c1 - c0
                    nc.vector.tensor_add(
                        ps[:, 0:n], ps[:, 0:n], yu_sb[:, c0:c1]
                    )
                    if gi == 0:
                        og_t = opool.tile([CC, 4 * CHUNK], BF, tag="og")
                        ostate["og"] = og_t
                    og = ostate["og"]
                    nc.scalar.activation(
                        og[:, c0 - g0 : c1 - g0],
                        ps[:, 0:n],
                        mybir.ActivationFunctionType.Relu,
                        bias=sh_sb[:],
                        scale=1.0,
                    )
                    if c1 == g1:  # last chunk of group -> store
                        oeng.dma_start(o_d[i, :, g0:g1], og[:, 0 : g1 - g0])

                conv_pass(xs1, xs2, wv_sb, emit_img)

    nc.compile()
    return nc


def _get_program():
    if "nc" not in _CACHE:
        _CACHE["nc"] = _build_program()
    return _CACHE["nc"]


def _pack_weights(wsc, base_ch):
    """lhsT pack [CC, 5*CC]: slot s rows j = tap_a weights for in-ch
    base_ch+j, rows 64+j = tap_b; skip identity folded into slot-1 upper."""
    wpk = np.zeros((CC, 5 * CC), np.float32)
    for s, (ta, tb, _, _) in enumerate(SLOTS):
        wpk[0:64, s * CC : (s + 1) * CC] = wsc[
            :, base_ch : base_ch + 64, ta[0] + 1, ta[1] + 1
        ].T
        if tb is not None:
            wpk[64:CC, s * CC : (s + 1) * CC] = wsc[
                :, base_ch : base_ch + 64, tb[0] + 1, tb[1] + 1
            ].T
    wpk[64:CC, 1 * CC + base_ch : 1 * CC + base_ch + 64] += np.eye(64, dtype=np.float32)
    wpk[64:CC, 4 * CC : 5 * CC] = wpk[0:64, 4 * CC : 5 * CC]
    return wpk.astype(BF16)


def _prep_inputs(u, v, conv_w, bn_gamma, bn_beta, bn_mean, bn_var):
    u = np.asarray(u, dtype=np.float32)
    v = np.asarray(v, dtype=np.float32)
    conv_w = np.asarray(conv_w, dtype=np.float32)
    scale = np.asarray(bn_gamma, np.float32) / np.sqrt(
        np.asarray(bn_var, np.float32) + EPS
    )
    shift = (np.asarray(bn_beta, np.float32) - np.asarray(bn_mean, np.float32) * scale)
    shift = shift.astype(np.float32).reshape(CC, 1)
    wsc = conv_w * scale[:, None, None, None]
    wu_host = _pack_weights(wsc, 0)
    wv_host = _pack_weights(wsc, 64)

    in_maps = []
    for m in range(N_CORES):
        b = m // 2
        s0 = (m % 2) * IMG
        u_pad = np.zeros((C1, HP, WP), np.float32)
        u_pad[:, 1 : 1 + H, 1 : 1 + W] = u[b, 0]
        v_pad = np.zeros((IMG, C2, HP, WP), np.float32)
        v_pad[:, :, 1 : 1 + H, 1 : 1 + W] = v[b, s0 : s0 + IMG]
        in_maps.append(
            {
                "u": u_pad.reshape(C1, NPX).astype(BF16),
                "v": v_pad.reshape(IMG, C2, NPX).astype(BF16),
                "wu": wu_host,
                "wv": wv_host,
                "shift": shift,
            }
        )
    return in_maps


def _run(inputs, trace=False):
    nc = _get_program()
    in_maps = _prep_inputs(**inputs)
    res = run_bass_kernel_spmd(nc, in_maps, list(range(N_CORES)), trace=trace)
    out = np.empty((B, 1, S, CC, H, W), np.float32)
    for m in range(N_CORES):
        b = m // 2
        s0 = (m % 2) * IMG
        o_pad = res.results[m]["o"].astype(np.float32).reshape(IMG, CC, H, WP)
        out[b, 0, s0 : s0 + IMG] = o_pad[:, :, :, 1 : 1 + W]
    return out, res


def kernel(u, v, conv_w, bn_gamma, bn_beta, bn_mean, bn_var):
    out, _ = _run(
        dict(
            u=u,
            v=v,
            conv_w=conv_w,
            bn_gamma=bn_gamma,
            bn_beta=bn_beta,
            bn_mean=bn_mean,
            bn_var=bn_var,
        )
    )
    return out


# revision 4
# speedup vs baseline: 1.4007x; 1.4007x over previous
"""CrossConv2d (concat -> 3x3 conv -> BN -> +skip -> ReLU) on 8 Trainium2 cores.

Data-parallel over (b*s)=32: 4 images per core, all sharing one u (same b).
Design vs the fp32r baseline:
  - bf16 everywhere (inputs, weights, outputs): FWL weight loads hide under
    matmul streaming (fp32r self-loads serially, ~128cy/matmul -> was
    LDWEIGHTS-bound at 281ns/MM), and HBM traffic halves. PSUM stays fp32.
  - u-sharing: the conv is linear in the concat input, so the 64 u-channel
    contribution y_u (incl. u's skip identity) is computed ONCE per core and
    added per-image via VectorE, cutting per-image contraction to 64 channels.
  - tap-pair packing: per-image 9 taps of K=64 are packed into 4 K=128
    matmuls + 1 K=64 matmul per 512-px chunk using two SBUF layouts per
    image: X1 = [v@0 ; v@+1col] and X2 = [v@0 ; v@+1row]; X2 is built from
    X1 by VectorE copies (in-partition shifts), not extra HBM reads.
  - the K=64 single-tap matmuls of adjacent chunks are row-tiled into PE
    halves (lhsT at base partition 0 vs 64) so each pair runs concurrently
    (~3ns apart) -- the X2 upper half holds the same v channels one row
    shifted, so the odd member reads X2[64:128] at col-WP and uses a
    duplicated weight block at rows 64:128.
  => ~208.7us measured vs 302.8-360.8us baseline (same-session 360.8).
"""

import numpy as np
import ml_dtypes

import concourse.bacc as bacc
import concourse.mybir as mybir
from concourse import tile
from concourse.bass_utils import run_bass_kernel_spmd

EPS = 1e-5
BF16 = ml_dtypes.bfloat16

B, S, C1, C2, H, W = 4, 8, 64, 64, 128, 128
CC = C1 + C2
N_CORES = 8
IMG = (B * S) // N_CORES  # 4
WP, HP = W + 2, H + 2     # padded width/height
NPX = HP * WP             # padded image pixels (16900)
NQ = H * WP               # output columns incl. junk pad cols (16640)
XW = NPX + 8              # X tile width (max col read = 16901)
CHUNK = 512               # one PSUM bank
NBLK = 16                 # DMA blocks per image half

F32 = mybir.dt.float32
BF = mybir.dt.bfloat16

# lhsT slots: (tap_a, tap_b|None, use_X2, rhs col offset)
# tap (dy,dx) at out q reads input flat q + off - 1 in lower-half layout;
# X1 upper = lower shifted +1 col, X2 upper = lower shifted +1 row (WP cols).
SLOTS = [
    ((-1, -1), (-1, 0), False, 0),
    ((0, -1), (0, 0), False, WP),       # center tap in upper -> skip identity
    ((1, -1), (1, 0), False, 2 * WP),
    ((-1, 1), (0, 1), True, 2),
    ((1, 1), None, True, 2 * WP + 2),   # single, K=64
]

_CACHE = {}


def _build_program():
    nc = bacc.Bacc(
        "TRN2", target_bir_lowering=False, debug=False, num_devices=N_CORES
    )
    u_d = nc.dram_tensor("u", [C1, NPX], BF, kind="ExternalInput")
    v_d = nc.dram_tensor("v", [IMG, C2, NPX], BF, kind="ExternalInput")
    wu_d = nc.dram_tensor("wu", [CC, 5 * CC], BF, kind="ExternalInput")
    wv_d = nc.dram_tensor("wv", [CC, 5 * CC], BF, kind="ExternalInput")
    sh_d = nc.dram_tensor("shift", [CC, 1], F32, kind="ExternalInput")
    o_d = nc.dram_tensor("o", [IMG, CC, NQ], BF, kind="ExternalOutput")

    blk = [(NPX * k // NBLK, NPX * (k + 1) // NBLK) for k in range(NBLK)]
    starts = [CHUNK * k for k in range(32)] + [32 * CHUNK]
    chunks = [(st, min(st + CHUNK, NQ)) for st in starts]

    with tile.TileContext(nc) as tc:
        with (
            tc.tile_pool(name="consts", bufs=1) as cpool,
            tc.tile_pool(name="ostrip", bufs=6) as opool,
            tc.tile_pool(name="psum", bufs=8, space="PSUM") as ppool,
        ):
            xa1 = cpool.tile([CC, XW], BF)
            xa2 = cpool.tile([CC, XW], BF)
            xb1 = cpool.tile([CC, XW], BF)
            xb2 = cpool.tile([CC, XW], BF)
            yu_sb = cpool.tile([CC, NQ], BF)
            wu_sb = cpool.tile([CC, 5 * CC], BF)
            wv_sb = cpool.tile([CC, 5 * CC], BF)
            sh_sb = cpool.tile([CC, 1], F32)

            # consts first: first matmul group only needs wu slot 0 + xa1 head
            nc.scalar.dma_start(wu_sb[:], wu_d[:])
            nc.scalar.dma_start(wv_sb[:], wv_d[:])
            nc.scalar.dma_start(sh_sb[:], sh_d[:])

            def fill_pads(x1, src_d):
                # lower col 0 and tail junk cols (read only by single-E at
                # q=16639); src row 0 of the padded image is all zeros.
                nc.scalar.dma_start(x1[0:64, 0:1], src_d[:, 0:1])
                nc.scalar.dma_start(x1[64:CC, NPX : NPX + 1], src_d[:, 0:1])
                nc.scalar.dma_start(x1[0:64, 1 + NPX : XW], src_d[:, 0 : XW - NPX - 1])

            def load_half(x1, j0, j1, src_lo, src_hi):
                # X1 lower: image at col 1+p ; X1 upper: image at col p
                nc.sync.dma_start(x1[0:64, 1 + j0 : 1 + j1], src_lo[:, j0:j1])
                nc.sync.dma_start(x1[64:CC, j0:j1], src_hi[:, j0:j1])

            def make_x2_lo(x1, x2, j0, j1):
                # X2 lower = X1 lower (identity, incl. pad cols on block 0 /
                # tail) — pure in-block copy.
                lo0, lo1 = (0 if j0 == 0 else 1 + j0), (XW if j1 == NPX else 1 + j1)
                nc.vector.tensor_copy(x2[0:64, lo0:lo1], x1[0:64, lo0:lo1])

            def make_x2_hi(x1, x2, j0, j1):
                # X2 upper col j = image flat j-1+WP = X1 upper col j+WP-1;
                # reads spill into the NEXT block, so callers issue this one
                # block behind the DMA.
                hi1 = min(j1, NPX + 2 - WP)
                if j0 < hi1:
                    nc.vector.tensor_copy(
                        x2[64:CC, j0:hi1], x1[64:CC, j0 + WP - 1 : hi1 + WP - 1]
                    )

            def load_image(x1, x2, src_d):
                fill_pads(x1, src_d)
                for j, (j0, j1) in enumerate(blk):
                    load_half(x1, j0, j1, src_d, src_d)
                    make_x2_lo(x1, x2, j0, j1)
                    if j > 0:
                        make_x2_hi(x1, x2, *blk[j - 1])
                make_x2_hi(x1, x2, *blk[-1])

            def conv_pass(x1, x2, w_sb, emit):
                """5-matmul conv over all chunks; emit(ps, c0, c1, gi, g0, g1)
                drains one chunk; gi = index of chunk in its group of 3."""
                for g0 in range(0, len(chunks), 4):
                    grp = chunks[g0 : g0 + 4]
                    pss = {}
                    for c0, _ in grp:
                        ps_g = ppool.tile([CC, CHUNK], F32, tag="ps")
                        pss[c0] = ps_g
                    for k, (_, tb, use_x2, off) in enumerate(SLOTS[:4]):
                        x = x2 if use_x2 else x1
                        for c0, c1 in grp:
                            nc.tensor.matmul(
                                pss[c0][0:CC, 0 : c1 - c0],
                                w_sb[0:CC, k * CC : (k + 1) * CC],
                                x[0:CC, c0 + off : c1 + off],
                                start=(k == 0),
                                stop=False,
                            )
                    off_e = SLOTS[4][3]
                    for ei, (c0, c1) in enumerate(grp):
                        if ei % 2 == 0:
                            nc.tensor.matmul(
                                pss[c0][0:CC, 0 : c1 - c0],
                                w_sb[0:64, 4 * CC : 5 * CC],
                                x2[0:64, c0 + off_e : c1 + off_e],
                                start=False,
                                stop=True,
                            )
                        else:
                            nc.tensor.matmul(
                                pss[c0][0:CC, 0 : c1 - c0],
                                w_sb[64:CC, 4 * CC : 5 * CC],
                                x2[64:CC, c0 + off_e - WP : c1 + off_e - WP],
                                start=False,
                                stop=True,
                            )
                    for gi, (c0, c1) in enumerate(grp):
                        emit(pss[c0], c0, c1, gi, grp[0][0], grp[-1][1])

            # ---- phase 0: y_u from U tiles (xa slots) ----
            load_image(xa1, xa2, u_d)

            def emit_yu(ps, c0, c1, gi, g0, g1):
                nc.scalar.activation(
                    yu_sb[:, c0:c1],
                    ps[:, 0 : c1 - c0],
                    mybir.ActivationFunctionType.Copy,
                )

            conv_pass(xa1, xa2, wu_sb, emit_yu)

            # ---- per-image passes ----
            for i in range(IMG):
                xs1, xs2 = (xb1, xb2) if i % 2 == 0 else (xa1, xa2)
                load_image(xs1, xs2, v_d[i])

                last_img = i == IMG - 1
                oeng = nc.sync if last_img else nc.gpsimd
                ostate = {}

                def emit_img(ps, c0, c1, gi, g0, g1, i=i, last_img=last_img,
                             oeng=oeng, ostate=ostate):
                    n = c1 - c0
                    nc.vector.tensor_add(
                        ps[:, 0:n], ps[:, 0:n], yu_sb[:, c0:c1]
                    )
                    if gi == 0:
                        og_t = opool.tile([CC, 4 * CHUNK], BF, tag="og")
                        ostate["og"] = og_t
                    og = ostate["og"]
                    nc.scalar.activation(
                        og[:, c0 - g0 : c1 - g0],
                        ps[:, 0:n],
                        mybir.ActivationFunctionType.Relu,
                        bias=sh_sb[:],
                        scale=1.0,
                    )
                    if c1 == g1:  # last chunk of group -> store
                        oeng.dma_start(o_d[i, :, g0:g1], og[:, 0 : g1 - g0])

                conv_pass(xs1, xs2, wv_sb, emit_img)

    nc.compile()
    return nc


def _get_program():
    if "nc" not in _CACHE:
        _CACHE["nc"] = _build_program()
    return _CACHE["nc"]


def _pack_weights(wsc, base_ch):
    """lhsT pack [CC, 5*CC]: slot s rows j = tap_a weights for in-ch
    base_ch+j, rows 64+j = tap_b; skip identity folded into slot-1 upper."""
    wpk = np.zeros((CC, 5 * CC), np.float32)
    for s, (ta, tb, _, _) in enumerate(SLOTS):
        wpk[0:64, s * CC : (s + 1) * CC] = wsc[
            :, base_ch : base_ch + 64, ta[0] + 1, ta[1] + 1
        ].T
        if tb is not None:
            wpk[64:CC, s * CC : (s + 1) * CC] = wsc[
                :, base_ch : base_ch + 64, tb[0] + 1, tb[1] + 1
            ].T
    wpk[64:CC, 1 * CC + base_ch : 1 * CC + base_ch + 64] += np.eye(64, dtype=np.float32)
    wpk[64:CC, 4 * CC : 5 * CC] = wpk[0:64, 4 * CC : 5 * CC]
    return wpk.astype(BF16)


def _prep_inputs(u, v, conv_w, bn_gamma, bn_beta, bn_mean, bn_var):
    u = np.asarray(u, dtype=np.float32)
    v = np.asarray(v, dtype=np.float32)
    conv_w = np.asarray(conv_w, dtype=np.float32)
    scale = np.asarray(bn_gamma, np.float32) / np.sqrt(
        np.asarray(bn_var, np.float32) + EPS
    )
    shift = (np.asarray(bn_beta, np.float32) - np.asarray(bn_mean, np.float32) * scale)
    shift = shift.astype(np.float32).reshape(CC, 1)
    wsc = conv_w * scale[:, None, None, None]
    wu_host = _pack_weights(wsc, 0)
    wv_host = _pack_weights(wsc, 64)

    in_maps = []
    for m in range(N_CORES):
        b = m // 2
        s0 = (m % 2) * IMG
        u_pad = np.zeros((C1, HP, WP), np.float32)
        u_pad[:, 1 : 1 + H, 1 : 1 + W] = u[b, 0]
        v_pad = np.zeros((IMG, C2, HP, WP), np.float32)
        v_pad[:, :, 1 : 1 + H, 1 : 1 + W] = v[b, s0 : s0 + IMG]
        in_maps.append(
            {
                "u": u_pad.reshape(C1, NPX).astype(BF16),
                "v": v_pad.reshape(IMG, C2, NPX).astype(BF16),
                "wu": wu_host,
                "wv": wv_host,
                "shift": shift,
            }
        )
    return in_maps


def _run(inputs, trace=False):
    nc = _get_program()
    in_maps = _prep_inputs(**inputs)
    res = run_bass_kernel_spmd(nc, in_maps, list(range(N_CORES)), trace=trace)
    out = np.empty((B, 1, S, CC, H, W), np.float32)
    for m in range(N_CORES):
        b = m // 2
        s0 = (m % 2) * IMG
        o_pad = res.results[m]["o"].astype(np.float32).reshape(IMG, CC, H, WP)
        out[b, 0, s0 : s0 + IMG] = o_pad[:, :, :, 1 : 1 + W]
    return out, res


def kernel(u, v, conv_w, bn_gamma, bn_beta, bn_mean, bn_var):
    out, _ = _run(
        dict(
            u=u,
            v=v,
            conv_w=conv_w,
            bn_gamma=bn_gamma,
            bn_beta=bn_beta,
            bn_mean=bn_mean,
            bn_var=bn_var,
        )
    )
    return out


# revision 6
# speedup vs baseline: 1.4715x; 1.0505x over previous
"""CrossConv2d (concat -> 3x3 conv -> BN -> +skip -> ReLU) on 8 Trainium2 cores.

Data-parallel over (b*s)=32: 4 images per core, all sharing one u (same b).
Design vs the fp32r baseline:
  - bf16 everywhere (inputs, weights, outputs): FWL weight loads hide under
    matmul streaming (fp32r self-loads serially, ~128cy/matmul -> was
    LDWEIGHTS-bound at 281ns/MM), and HBM traffic halves. PSUM stays fp32.
  - u-sharing: the conv is linear in the concat input, so the 64 u-channel
    contribution y_u (incl. u's skip identity) is computed ONCE per core and
    added per-image via VectorE, cutting per-image contraction to 64 channels.
  - tap-pair packing: per-image 9 taps of K=64 are packed into 4 K=128
    matmuls + 1 K=64 matmul per 512-px chunk using two SBUF layouts per
    image: X1 = [v@0 ; v@+1col] and X2 = [v@0 ; v@+1row]; X2 is built from
    X1 by VectorE copies (in-partition shifts), not extra HBM reads.
  - the K=64 single-tap matmuls of adjacent chunks are row-tiled into PE
    halves (lhsT at base partition 0 vs 64) so each pair runs concurrently
    (~3ns apart) -- the X2 upper half holds the same v channels one row
    shifted, so the odd member reads X2[64:128] at col-WP and uses a
    duplicated weight block at rows 64:128.
  - the last image's final PSUM groups shrink to 2,2,1 chunks so the
    closing add->relu->store chain drains sooner after the last matmul.
  => 205.2us measured vs 302.8-360.8us baseline (same-session 360.8).
"""

import numpy as np
import ml_dtypes

import concourse.bacc as bacc
import concourse.mybir as mybir
from concourse import tile
from concourse.bass_utils import run_bass_kernel_spmd

EPS = 1e-5
BF16 = ml_dtypes.bfloat16

B, S, C1, C2, H, W = 4, 8, 64, 64, 128, 128
CC = C1 + C2
N_CORES = 8
IMG = (B * S) // N_CORES  # 4
WP, HP = W + 2, H + 2     # padded width/height
NPX = HP * WP             # padded image pixels (16900)
NQ = H * WP               # output columns incl. junk pad cols (16640)
XW = NPX + 8              # X tile width (max col read = 16901)
CHUNK = 512               # one PSUM bank
NBLK = 16                 # DMA blocks per image half

F32 = mybir.dt.float32
BF = mybir.dt.bfloat16

# lhsT slots: (tap_a, tap_b|None, use_X2, rhs col offset)
# tap (dy,dx) at out q reads input flat q + off - 1 in lower-half layout;
# X1 upper = lower shifted +1 col, X2 upper = lower shifted +1 row (WP cols).
SLOTS = [
    ((-1, -1), (-1, 0), False, 0),
    ((0, -1), (0, 0), False, WP),       # center tap in upper -> skip identity
    ((1, -1), (1, 0), False, 2 * WP),
    ((-1, 1), (0, 1), True, 2),
    ((1, 1), None, True, 2 * WP + 2),   # single, K=64
]

_CACHE = {}


def _build_program():
    nc = bacc.Bacc(
        "TRN2", target_bir_lowering=False, debug=False, num_devices=N_CORES
    )
    u_d = nc.dram_tensor("u", [C1, NPX], BF, kind="ExternalInput")
    v_d = nc.dram_tensor("v", [IMG, C2, NPX], BF, kind="ExternalInput")
    wu_d = nc.dram_tensor("wu", [CC, 5 * CC], BF, kind="ExternalInput")
    wv_d = nc.dram_tensor("wv", [CC, 5 * CC], BF, kind="ExternalInput")
    sh_d = nc.dram_tensor("shift", [CC, 1], F32, kind="ExternalInput")
    o_d = nc.dram_tensor("o", [IMG, CC, NQ], BF, kind="ExternalOutput")

    blk = [(NPX * k // NBLK, NPX * (k + 1) // NBLK) for k in range(NBLK)]
    starts = [CHUNK * k for k in range(32)] + [32 * CHUNK]
    chunks = [(st, min(st + CHUNK, NQ)) for st in starts]

    with tile.TileContext(nc) as tc:
        with (
            tc.tile_pool(name="consts", bufs=1) as cpool,
            tc.tile_pool(name="ostrip", bufs=6) as opool,
            tc.tile_pool(name="psum", bufs=8, space="PSUM") as ppool,
        ):
            xa1 = cpool.tile([CC, XW], BF)
            xa2 = cpool.tile([CC, XW], BF)
            xb1 = cpool.tile([CC, XW], BF)
            xb2 = cpool.tile([CC, XW], BF)
            yu_sb = cpool.tile([CC, NQ], BF)
            wu_sb = cpool.tile([CC, 5 * CC], BF)
            wv_sb = cpool.tile([CC, 5 * CC], BF)
            sh_sb = cpool.tile([CC, 1], F32)

            # consts first: first matmul group only needs wu slot 0 + xa1 head
            nc.scalar.dma_start(wu_sb[:], wu_d[:])
            nc.scalar.dma_start(wv_sb[:], wv_d[:])
            nc.scalar.dma_start(sh_sb[:], sh_d[:])

            def fill_pads(x1, src_d):
                # lower col 0 and tail junk cols (read only by single-E at
                # q=16639); src row 0 of the padded image is all zeros.
                nc.scalar.dma_start(x1[0:64, 0:1], src_d[:, 0:1])
                nc.scalar.dma_start(x1[64:CC, NPX : NPX + 1], src_d[:, 0:1])
                nc.scalar.dma_start(x1[0:64, 1 + NPX : XW], src_d[:, 0 : XW - NPX - 1])

            def load_half(x1, j0, j1, src_lo, src_hi):
                # X1 lower: image at col 1+p ; X1 upper: image at col p
                nc.sync.dma_start(x1[0:64, 1 + j0 : 1 + j1], src_lo[:, j0:j1])
                nc.sync.dma_start(x1[64:CC, j0:j1], src_hi[:, j0:j1])

            def make_x2_lo(x1, x2, j0, j1):
                # X2 lower = X1 lower (identity, incl. pad cols on block 0 /
                # tail) — pure in-block copy.
                lo0, lo1 = (0 if j0 == 0 else 1 + j0), (XW if j1 == NPX else 1 + j1)
                nc.vector.tensor_copy(x2[0:64, lo0:lo1], x1[0:64, lo0:lo1])

            def make_x2_hi(x1, x2, j0, j1):
                # X2 upper col j = image flat j-1+WP = X1 upper col j+WP-1;
                # reads spill into the NEXT block, so callers issue this one
                # block behind the DMA.
                hi1 = min(j1, NPX + 2 - WP)
                if j0 < hi1:
                    nc.vector.tensor_copy(
                        x2[64:CC, j0:hi1], x1[64:CC, j0 + WP - 1 : hi1 + WP - 1]
                    )

            def load_image(x1, x2, src_d):
                fill_pads(x1, src_d)
                for j, (j0, j1) in enumerate(blk):
                    load_half(x1, j0, j1, src_d, src_d)
                    make_x2_lo(x1, x2, j0, j1)
                    if j > 0:
                        make_x2_hi(x1, x2, *blk[j - 1])
                make_x2_hi(x1, x2, *blk[-1])

            def conv_pass(x1, x2, w_sb, emit, tail_split=False):
                """5-matmul conv over all chunks; emit(ps, c0, c1, gi, g0, g1)
                drains one chunk. tail_split shortens the final drain chain
                (last image): groups of 4 except 2,2,1 at the end."""
                if tail_split:
                    bounds = list(range(0, 28, 4)) + [28, 30, 32]
                else:
                    bounds = list(range(0, len(chunks), 4))
                ext = bounds[1:] + [len(chunks)]
                for g0, ge in zip(bounds, ext):
                    grp = chunks[g0:ge]
                    pss = {}
                    for c0, _ in grp:
                        ps_g = ppool.tile([CC, CHUNK], F32, tag="ps")
                        pss[c0] = ps_g
                    for k, (_, tb, use_x2, off) in enumerate(SLOTS[:4]):
                        x = x2 if use_x2 else x1
                        for c0, c1 in grp:
                            nc.tensor.matmul(
                                pss[c0][0:CC, 0 : c1 - c0],
                                w_sb[0:CC, k * CC : (k + 1) * CC],
                                x[0:CC, c0 + off : c1 + off],
                                start=(k == 0),
                                stop=False,
                            )
                    off_e = SLOTS[4][3]
                    for ei, (c0, c1) in enumerate(grp):
                        if ei % 2 == 0:
                            nc.tensor.matmul(
                                pss[c0][0:CC, 0 : c1 - c0],
                                w_sb[0:64, 4 * CC : 5 * CC],
                                x2[0:64, c0 + off_e : c1 + off_e],
                                start=False,
                                stop=True,
                            )
                        else:
                            nc.tensor.matmul(
                                pss[c0][0:CC, 0 : c1 - c0],
                                w_sb[64:CC, 4 * CC : 5 * CC],
                                x2[64:CC, c0 + off_e - WP : c1 + off_e - WP],
                                start=False,
                                stop=True,
                            )
                    for gi, (c0, c1) in enumerate(grp):
                        emit(pss[c0], c0, c1, gi, grp[0][0], grp[-1][1])

            # ---- phase 0: y_u from U tiles (xa slots) ----
            load_image(xa1, xa2, u_d)

            def emit_yu(ps, c0, c1, gi, g0, g1):
                nc.scalar.activation(
                    yu_sb[:, c0:c1],
                    ps[:, 0 : c1 - c0],
                    mybir.ActivationFunctionType.Copy,
                )

            conv_pass(xa1, xa2, wu_sb, emit_yu)

            # ---- per-image passes ----
            for i in range(IMG):
                xs1, xs2 = (xb1, xb2) if i % 2 == 0 else (xa1, xa2)
                load_image(xs1, xs2, v_d[i])

                last_img = i == IMG - 1
                oeng = nc.sync if last_img else nc.gpsimd
                ostate = {}

                def emit_img(ps, c0, c1, gi, g0, g1, i=i, last_img=last_img,
                             oeng=oeng, ostate=ostate):
                    n = c1 - c0
                    nc.vector.tensor_add(
                        ps[:, 0:n], ps[:, 0:n], yu_sb[:, c0:c1]
                    )
                    if gi == 0:
                        og_t = opool.tile([CC, 4 * CHUNK], BF, tag="og")
                        ostate["og"] = og_t
                    og = ostate["og"]
                    nc.scalar.activation(
                        og[:, c0 - g0 : c1 - g0],
                        ps[:, 0:n],
                        mybir.ActivationFunctionType.Relu,
                        bias=sh_sb[:],
                        scale=1.0,
                    )
                    if c1 == g1:  # last chunk of group -> store
                        oeng.dma_start(o_d[i, :, g0:g1], og[:, 0 : g1 - g0])

                conv_pass(xs1, xs2, wv_sb, emit_img, tail_split=last_img)

    nc.compile()
    return nc


def _get_program():
    if "nc" not in _CACHE:
        _CACHE["nc"] = _build_program()
    return _CACHE["nc"]


def _pack_weights(wsc, base_ch):
    """lhsT pack [CC, 5*CC]: slot s rows j = tap_a weights for in-ch
    base_ch+j, rows 64+j = tap_b; skip identity folded into slot-1 upper."""
    wpk = np.zeros((CC, 5 * CC), np.float32)
    for s, (ta, tb, _, _) in enumerate(SLOTS):
        wpk[0:64, s * CC : (s + 1) * CC] = wsc[
            :, base_ch : base_ch + 64, ta[0] + 1, ta[1] + 1
        ].T
        if tb is not None:
            wpk[64:CC, s * CC : (s + 1) * CC] = wsc[
                :, base_ch : base_ch + 64, tb[0] + 1, tb[1] + 1
            ].T
    wpk[64:CC, 1 * CC + base_ch : 1 * CC + base_ch + 64] += np.eye(64, dtype=np.float32)
    wpk[64:CC, 4 * CC : 5 * CC] = wpk[0:64, 4 * CC : 5 * CC]
    return wpk.astype(BF16)


def _prep_inputs(u, v, conv_w, bn_gamma, bn_beta, bn_mean, bn_var):
    u = np.asarray(u, dtype=np.float32)
    v = np.asarray(v, dtype=np.float32)
    conv_w = np.asarray(conv_w, dtype=np.float32)
    scale = np.asarray(bn_gamma, np.float32) / np.sqrt(
        np.asarray(bn_var, np.float32) + EPS
    )
    shift = (np.asarray(bn_beta, np.float32) - np.asarray(bn_mean, np.float32) * scale)
    shift = shift.astype(np.float32).reshape(CC, 1)
    wsc = conv_w * scale[:, None, None, None]
    wu_host = _pack_weights(wsc, 0)
    wv_host = _pack_weights(wsc, 64)

    in_maps = []
    for m in range(N_CORES):
        b = m // 2
        s0 = (m % 2) * IMG
        u_pad = np.zeros((C1, HP, WP), np.float32)
        u_pad[:, 1 : 1 + H, 1 : 1 + W] = u[b, 0]
        v_pad = np.zeros((IMG, C2, HP, WP), np.float32)
        v_pad[:, :, 1 : 1 + H, 1 : 1 + W] = v[b, s0 : s0 + IMG]
        in_maps.append(
            {
                "u": u_pad.reshape(C1, NPX).astype(BF16),
                "v": v_pad.reshape(IMG, C2, NPX).astype(BF16),
                "wu": wu_host,
                "wv": wv_host,
                "shift": shift,
            }
        )
    return in_maps


def _run(inputs, trace=False):
    nc = _get_program()
    in_maps = _prep_inputs(**inputs)
    res = run_bass_kernel_spmd(nc, in_maps, list(range(N_CORES)), trace=trace)
    out = np.empty((B, 1, S, CC, H, W), np.float32)
    for m in range(N_CORES):
        b = m // 2
        s0 = (m % 2) * IMG
        o_pad = res.results[m]["o"].astype(np.float32).reshape(IMG, CC, H, WP)
        out[b, 0, s0 : s0 + IMG] = o_pad[:, :, :, 1 : 1 + W]
    return out, res


def kernel(u, v, conv_w, bn_gamma, bn_beta, bn_mean, bn_var):
    out, _ = _run(
        dict(
            u=u,
            v=v,
            conv_w=conv_w,
            bn_gamma=bn_gamma,
            bn_beta=bn_beta,
            bn_mean=bn_mean,
            bn_var=bn_var,
        )
    )
    return out
